# revision 59
# baseline (speedup 1.0000x reference)
"""Trainium2 Bass kernel for MultiLinearAttention (causal linear attention).

Reference computation (per head h, feature map phi(u) = elu(u)+1):
    q = phi(x_h @ Wq_h), k = phi(x_h @ Wk_h), v = x_h @ Wv_h
    y_t = (q_t . sum_{s<=t} k_s v_s^T) / (q_t . sum_{s<=t} k_s + eps)
    out = concat_h(y_h) @ Wp

Sharding: 16 heads / 8 cores = 2 heads per core, all 4 batches per core.
Wp is folded per-head into the v projection (W'_h = Wv_h @ Wp_h), so each
core produces a partial output summed on the host.  The device emits raw
per-head numerator/denominator pairs; the divide + head-sum happens on the
host (removes the whole y-epilogue from the device's vector engines).

Device algorithm: chunked causal linear attention, chunk C=128:
    A^T = K_chunk Q_chunk^T (per head), masked to s<=t
    num = A_m^T V'aug + Q^T S_aug   (aug col of V' is ones -> den)
    S_aug += K_chunk^T V'aug
phi(u) = elu(u)+1 = min(max(u+1, 1), exp(u)) computed from the raw
projection u: exp on Scalar; max(u+1,1) on Vector (depends only on u,
so it overlaps the exp); final min on Vector.
PSUM accumulation uses the has_written bits (first matmul per bank
start=True clears them; later matmuls overwrite unwritten regions and
accumulate over written ones), so no preset matmuls are needed.
K^T (time-major k for the state update) comes from PE transposes into
a bf16 bitcast sub-region of an A bank (saves a PSUM bank so u can be
double-buffered within the 8-bank budget).

Two batches are processed per instruction ("pair batching"); q/k
projections are additionally merged across the pair via an interleaved
[chunk, batch] x layout so each is a single N=256 matmul.  The v
projections of BOTH pairs are hoisted to a per-chunk preamble (one
shared PSUM bank, one fused V'aug copy): they depend only on x, so
they fill PE bubbles at chunk starts and unblock the A.V matmuls
earlier.  The device ships raw per-head num/den pairs (grouped 4
bodies per DMA so each partition moves 2080 contiguous bytes — thin
520B/partition DMAs run ~15GB/s); divide + head-sum happen on the host.
"""

import os
import sys

import numpy as np

for _p in ("/root/.axon_site/_ro/trn_rl_repo", "/opt/trn_rl_repo", "/opt/pypackages"):
    if os.path.isdir(_p) and _p not in sys.path:
        sys.path.append(_p)

import ml_dtypes

B, S, D = 4, 4096, 1024
H, HD, O = 16, 64, 64
C = 128                  # chunk length
NCORE = 8
HPC = H // NCORE         # heads per core
NCHUNK = S // C

USE_BF16 = True

_CACHE = {}


def _build_program(nchunk=NCHUNK):
    import concourse.mybir as mybir
    from concourse import bacc
    from concourse.tile import TileContext

    fp32 = mybir.dt.float32
    cdt = mybir.dt.bfloat16 if USE_BF16 else fp32
    Alu = mybir.AluOpType
    Act = mybir.ActivationFunctionType

    nc = bacc.Bacc()
    # x interleaved per pair: [pr, 128, nchunk, 2, C] (chunk-major, batch
    # within chunk) so one matmul's moving operand covers both batches.
    xT_h = nc.declare_dram_parameter("xT", [2, 128, nchunk * 2 * C], cdt,
                                     isOutput=False)
    wq_h = nc.declare_dram_parameter("wq", [128, 128], cdt, isOutput=False)
    wk_h = nc.declare_dram_parameter("wk", [128, 128], cdt, isOutput=False)
    wv_h = nc.declare_dram_parameter("wv", [128, 128], cdt, isOutput=False)
    mask_h = nc.declare_dram_parameter("mask2", [128, 512], cdt, isOutput=False)
    ident_h = nc.declare_dram_parameter("ident", [128, 128], cdt, isOutput=False)
    ones_h = nc.declare_dram_parameter("ones", [1, 512], cdt, isOutput=False)
    zer_h = nc.declare_dram_parameter("zer", [1, 512], cdt, isOutput=False)
    # output grouped 2 chunks (4 bodies) per DMA so each partition ships
    # 2080 contiguous bytes (big packets; 520B/partition DMAs run ~15GB/s)
    out_h = nc.declare_dram_parameter("out", [nchunk // 2, 128, 4 * 260], cdt,
                                      isOutput=True)

    NXQ = 8       # x streamed in slices per pair for early start
    WORKBUFS = 5  # work pool depth (also gates the one-time vones memsets)

    with TileContext(nc) as tc:
        with (
            tc.tile_pool(name="consts", bufs=1) as consts,
            tc.tile_pool(name="work", bufs=WORKBUFS) as work,
            tc.tile_pool(name="st_sb", bufs=2) as st_sb,
            tc.tile_pool(name="osb", bufs=4) as osb,
            tc.tile_pool(name="pu", bufs=2, space="PSUM") as pu,
            tc.tile_pool(name="pa", bufs=1, space="PSUM") as pa,
            tc.tile_pool(name="pvk", bufs=1, space="PSUM") as pvk,
            tc.tile_pool(name="pnum", bufs=1, space="PSUM") as pnum,
            tc.tile_pool(name="pst", bufs=1, space="PSUM") as pst,
        ):
            # ---- constants + x into SBUF ----
            # Issue order matters: the first x slices and the weights gate
            # the first body, so they go before the other consts.
            wq = consts.tile([128, 128], cdt)
            wk = consts.tile([128, 128], cdt)
            wv = consts.tile([128, 128], cdt)
            mask2 = consts.tile([128, 512], cdt)
            ident = consts.tile([128, 128], cdt)
            ones = consts.tile([1, 512], cdt)
            zer = consts.tile([1, 512], cdt)
            xsb = [consts.tile([128, nchunk * 2 * C], cdt, name=f"xsb{pr}")
                   for pr in range(2)]
            qn = nchunk * 2 * C // NXQ
            nc.sync.dma_start(xsb[0][:, 0:qn], xT_h[0, :, 0:qn])
            nc.sync.dma_start(wv, wv_h[:, :])
            nc.sync.dma_start(wq, wq_h[:, :])
            nc.sync.dma_start(wk, wk_h[:, :])
            nc.sync.dma_start(xsb[1][:, 0:qn], xT_h[1, :, 0:qn])
            nc.sync.dma_start(mask2, mask_h[:, :])
            nc.sync.dma_start(ident, ident_h[:, :])
            nc.sync.dma_start(ones, ones_h[:, :])
            nc.sync.dma_start(zer, zer_h[:, :])
            for q in range(1, NXQ):
                for pr in range(2):
                    nc.sync.dma_start(xsb[pr][:, q * qn:(q + 1) * qn],
                                      xT_h[pr, :, q * qn:(q + 1) * qn])

            # persistent per-pair state PSUM: each [128, 260] with, per
            # j-block of 130 cols, head blocks [64h:64h+64, 65hp:65hp+65]
            # (hp = h ^ j).  One-time zero write sets data AND has_written
            # bits so later state matmuls accumulate; off-diag stays 0.
            st_ps = [
                pst.tile([128, 260], fp32, name="stA"),
                pst.tile([128, 260], fp32, name="stB"),
            ]
            for stp in st_ps:
                nc.tensor.matmul(stp, ones[:, 0:128], zer[:, 0:260],
                                 start=True, stop=False, skip_group_check=True)

            s01_prev = [None, None]

            for i in range(nchunk):
                csl = slice(i * 2 * C, (i + 1) * 2 * C)       # both batches
                bsl = [slice(i * 2 * C + j * C, i * 2 * C + (j + 1) * C)
                       for j in range(2)]                     # per batch

                # ---- v projections + V'aug for BOTH pairs (fused) ----
                # vk (1 bank): [v'(b0)|v'(b1)|v'(b2)|v'(b3)].  v needs only
                # x, so these dependency-free matmuls also fill PE bubbles
                # at chunk starts; the fused vaug copy halves its fixed cost.
                vk = pvk.tile([128, 512], fp32, name="vk")
                for vpr in range(2):
                    for j in range(2):
                        n = 2 * vpr + j
                        nc.tensor.matmul(vk[:, 128 * n:128 * (n + 1)],
                                         xsb[vpr][:, bsl[j]], wv,
                                         start=(n == 0), stop=(n == 3),
                                         skip_group_check=True)
                vaug = work.tile([128, 520], cdt, name="vaug")
                vsrc = vk.rearrange("p (g c) -> p g c", c=64)
                vdst = vaug.rearrange("p (g c) -> p g c", c=65)[:, :, 0:64]
                nc.scalar.copy(vdst, vsrc)
                if i < WORKBUFS:
                    vones = vaug.rearrange(
                        "p (g c) -> p g c", c=65)[:, :, 64:65]
                    nc.gpsimd.memset(vones, 1.0)

                for pr in range(2):
                    stp = st_ps[pr]
                    xp = xsb[pr]
                    V = 260 * pr     # this pair's vaug column offset

                    # ---------------- PE: projections ----------------
                    # u layout: [q(b0) | q(b1) | k(b0) | k(b1)] each [128,128]
                    u = pu.tile([128, 512], fp32, name="u")
                    nc.tensor.matmul(u[:, 0:256], wq, xp[:, csl],
                                     start=True, stop=False,
                                     skip_group_check=True)
                    nc.tensor.matmul(u[:, 256:512], wk, xp[:, csl],
                                     start=False, stop=True,
                                     skip_group_check=True)

                    # ------- phi = elu(u)+1 = min(exp(u), 1) + relu(u) -------
                    # (u<=0 -> exp(u)+0; u>0 -> 1+u).  The min runs as a
                    # cheap pure-bf16 tensor_scalar (2x DVE rate); the single
                    # fp32 PSUM read of u rides the fused relu+add STT.
                    e2 = work.tile([128, 512], cdt, name="e2")
                    nc.scalar.activation(e2, u, Act.Exp)
                    m2 = work.tile([128, 512], cdt, name="m2")
                    nc.vector.tensor_scalar_min(m2, e2, 1.0)
                    phi2 = work.tile([128, 512], cdt, name="phi2")
                    nc.vector.scalar_tensor_tensor(
                        phi2, u, 0.0, m2, Alu.max, Alu.add)

                    # ---------------- A^T = K Q^T per (b, h) ----------------
                    # One PSUM bank per head: all matmuls writing a given bank
                    # must read operands from the same base partition (HW).
                    # Bank A additionally hosts the k-transpose region (bf16
                    # via bitcast) — its writers also read base-0 operands,
                    # and their start=True bit-clears don't disturb ah data.
                    bankA = pa.tile([128, 512], fp32, name="bankA")
                    bankB = pa.tile([128, 512], fp32, name="bankB")
                    ah = [bankA[:, 0:256], bankB[:, 0:256]]
                    knp = bankA[:, 256:384].bitcast(cdt)   # [128, 256] bf16
                    for j in range(2):
                        qq = phi2[:, 128 * j:128 * (j + 1)]
                        kk = phi2[:, 256 + 128 * j:256 + 128 * (j + 1)]
                        for h in range(2):
                            es = slice(64 * h, 64 * (h + 1))
                            nc.tensor.matmul(
                                ah[h][:, 128 * j:128 * (j + 1)],
                                kk[es, :], qq[es, :],
                                start=(j == 0), stop=(j == 1),
                                skip_group_check=True)

                    # masked A -> SBUF; layout [b0h0 | b1h0 | b0h1 | b1h1]
                    am2 = work.tile([128, 512], cdt, name="am2")
                    nc.vector.tensor_tensor(am2[:, 0:256], ah[0],
                                            mask2[:, 0:256], Alu.mult)
                    nc.vector.tensor_tensor(am2[:, 256:512], ah[1],
                                            mask2[:, 256:512], Alu.mult)

                    # k transposed to [t, e] per j via PE transpose (skipped
                    # on the final chunk — nothing consumes the state after)
                    if i < nchunk - 1:
                        for j in range(2):
                            nc.tensor.transpose(
                                knp[:, 128 * j:128 * (j + 1)],
                                phi2[:, 256 + 128 * j:256 + 128 * (j + 1)],
                                ident)
                        knat2 = work.tile([128, 256], cdt, name="knat2")
                        nc.vector.tensor_copy(knat2, knp)

                    # ---------------- num = A_m^T Vaug + Q^T S ----------------
                    # For b1, head blocks are stored swapped (h1 first) so the
                    # h1 state block (output partition offset 64) lands at a
                    # column where its AP stays within one PSUM bank. Heads
                    # are summed at the end, so block identity is positional.
                    num = pnum.tile([128, 260], fp32, name="num")
                    first = True
                    for j in range(2):
                        for h in range(2):
                            hp = h ^ j  # head's positional slot
                            reg = slice(130 * j + 65 * hp,
                                        130 * j + 65 * (hp + 1))
                            va = vaug[:, V + 130 * j + 65 * h:
                                      V + 130 * j + 65 * (h + 1)]
                            nc.tensor.matmul(
                                num[:, reg],
                                am2[:, 256 * h + 128 * j:256 * h + 128 * (j + 1)],
                                va, start=first,
                                stop=(i == 0 and j == 1 and h == 1),
                                skip_group_check=True)
                            first = False
                        if i > 0:
                            # both heads at once: K=128 with block-diag state
                            sp = s01_prev[pr]
                            nc.tensor.matmul(
                                num[:, 130 * j:130 * (j + 1)],
                                phi2[:, 128 * j:128 * (j + 1)],
                                sp[:, 130 * j:130 * (j + 1)],
                                start=False, stop=(j == 1),
                                skip_group_check=True)

                    # ---------------- emit num/den to HBM (host divides) -----
                    # Emitted before the state update so the scalar queue
                    # frees the num bank early (the next body's AV matmuls
                    # wait on it); 4 bodies staged per osb tile, one fat DMA
                    # per 2 chunks.
                    if i % 2 == 0 and pr == 0:
                        numsb = osb.tile([128, 4 * 260], cdt, name="numsb")
                    off = ((i % 2) * 2 + pr) * 260
                    nc.scalar.copy(numsb[:, off:off + 260], num)
                    if i % 2 == 1 and pr == 1:
                        nc.sync.dma_start(out_h[i // 2], numsb)

                    # ---------------- state update (diag blocks only) --------
                    # Per-head matmuls with base-0 operands; h1 writes at
                    # output partition offset 64. Off-diag blocks stay zero.
                    # Skipped on the final chunk (state is never read again).
                    if i < nchunk - 1:
                        for j in range(2):
                            for h in range(2):
                                hp = h ^ j
                                nc.tensor.matmul(
                                    stp[64 * h:64 * (h + 1),
                                        130 * j + 65 * hp:
                                        130 * j + 65 * (hp + 1)],
                                    knat2[:, 128 * j + 64 * h:
                                          128 * j + 64 * (h + 1)],
                                    vaug[:, V + 130 * j + 65 * h:
                                         V + 130 * j + 65 * (h + 1)],
                                    start=False, stop=False,
                                    skip_group_check=True)

                        s01 = st_sb.tile([128, 260], cdt, name="s01")
                        nc.scalar.copy(s01, stp)
                        s01_prev[pr] = s01

    nc.finalize()
    return nc


def _host_prep(x, Wq, Wk, Wv, Wp):
    """Shard inputs per core; returns in_maps list."""
    x = np.asarray(x, dtype=np.float32)
    Wq = np.asarray(Wq, dtype=np.float32)
    Wk = np.asarray(Wk, dtype=np.float32)
    Wv = np.asarray(Wv, dtype=np.float32)
    Wp = np.asarray(Wp, dtype=np.float32)
    ndt = ml_dtypes.bfloat16 if USE_BF16 else np.float32

    mask = np.triu(np.ones((C, C), np.float32))
    mask2 = np.tile(mask, (1, 4)).astype(ndt)          # [128, 512]
    ident = np.eye(128, dtype=np.float32).astype(ndt)
    ones = np.ones((1, 512), np.float32).astype(ndt)

    in_maps = []
    for c in range(NCORE):
        h0 = HPC * c
        xs = x[:, :, 64 * h0:64 * (h0 + HPC)]          # [B, S, 128]
        # [pr, 128, nchunk, 2, C]: chunk-major with the pair's two batches
        # interleaved per chunk.
        xT = xs.transpose(0, 2, 1).reshape(2, 2, 128, NCHUNK, C)
        xT = np.ascontiguousarray(xT.transpose(0, 2, 3, 1, 4)).reshape(
            2, 128, NCHUNK * 2 * C).astype(ndt)
        wq_bd = np.zeros((128, 128), np.float32)
        wk_bd = np.zeros((128, 128), np.float32)
        wv_bd = np.zeros((128, 128), np.float32)
        for j in range(HPC):
            h = h0 + j
            sl = slice(64 * j, 64 * (j + 1))
            wq_bd[sl, sl] = Wq[h]
            wk_bd[sl, sl] = Wk[h]
            wv_bd[sl, sl] = Wv[h] @ Wp[64 * h:64 * (h + 1), :]
        in_maps.append({
            "xT": xT,
            "wq": wq_bd.astype(ndt),
            "wk": wk_bd.astype(ndt),
            "wv": wv_bd.astype(ndt),
            "mask2": mask2,
            "ident": ident,
            "ones": ones,
            "zer": np.zeros((1, 512), np.float32).astype(ndt),
        })
    return in_maps


def get_program():
    if "nc" not in _CACHE:
        _CACHE["nc"] = _build_program()
    return _CACHE["nc"]


def run_spmd(in_maps, **kwargs):
    from concourse.bass_utils import run_bass_kernel_spmd
    nc = get_program()
    return run_bass_kernel_spmd(nc, in_maps, list(range(NCORE)), **kwargs)


def kernel(x, Wq, Wk, Wv, Wp):
    in_maps = _host_prep(x, Wq, Wk, Wv, Wp)
    res = run_spmd(in_maps)
    out = np.zeros((B, S, O), np.float32)
    for c in range(NCORE):
        # raw [ngrp, t, il, pr, j, slot, col]: chunk i = 4*g + il; per
        # j-block two head slots of [64 num | 1 den]; head order within a
        # block is irrelevant because the heads are summed.
        raw = np.asarray(res.results[c]["out"], dtype=np.float32)
        raw = raw.reshape(NCHUNK // 2, 128, 2, 2, 2, 2, 65)
        y = (raw[..., 0:64] / raw[..., 64:65]).sum(axis=5)
        # y: [g, t, il, pr, j, 64] -> b = 2*pr + j, token 128*(2g+il) + t
        out += y.transpose(3, 4, 0, 2, 1, 5).reshape(B, S, O)
    return out


# revision 60
# speedup vs baseline: 1.3245x; 1.3245x over previous
"""Trainium2 Bass kernel for MultiLinearAttention (causal linear attention).

Reference computation (per head h, feature map phi(u) = elu(u)+1):
    q = phi(x_h @ Wq_h), k = phi(x_h @ Wk_h), v = x_h @ Wv_h
    y_t = (q_t . sum_{s<=t} k_s v_s^T) / (q_t . sum_{s<=t} k_s + eps)
    out = concat_h(y_h) @ Wp

Sharding: 16 heads / 8 cores = 2 heads per core, all 4 batches per core.
Wp is folded per-head into the v projection (W'_h = Wv_h @ Wp_h), so each
core produces a partial output summed on the host.  The device emits raw
per-head numerator/denominator pairs; the divide + head-sum happens on the
host (removes the whole y-epilogue from the device's vector engines).

Device algorithm: chunked causal linear attention, chunk C=128:
    A^T = K_chunk Q_chunk^T (per head), masked to s<=t
    num = A_m^T V'aug + Q^T S_aug   (aug col of V' is ones -> den)
    S_aug += K_chunk^T V'aug
phi(u) = elu(u)+1 = min(max(u+1, 1), exp(u)) computed from the raw
projection u: exp on Scalar; max(u+1,1) on Vector (depends only on u,
so it overlaps the exp); final min on Vector.
PSUM accumulation uses the has_written bits (first matmul per bank
start=True clears them; later matmuls overwrite unwritten regions and
accumulate over written ones), so no preset matmuls are needed.
K^T (time-major k for the state update) comes from PE transposes into
a bf16 bitcast sub-region of an A bank (saves a PSUM bank so u can be
double-buffered within the 8-bank budget).

Two batches are processed per instruction ("pair batching"); q/k
projections are additionally merged across the pair via an interleaved
[chunk, batch] x layout so each is a single N=256 matmul.  The v
projections of BOTH pairs are hoisted to a per-chunk preamble (one
shared PSUM bank, one fused V'aug copy): they depend only on x, so
they fill PE bubbles at chunk starts and unblock the A.V matmuls
earlier.  The device ships raw per-head num/den pairs (grouped 4
bodies per DMA so each partition moves 2080 contiguous bytes — thin
520B/partition DMAs run ~15GB/s); divide + head-sum happen on the host.
"""

import os
import sys

import numpy as np

for _p in ("/root/.axon_site/_ro/trn_rl_repo", "/opt/trn_rl_repo", "/opt/pypackages"):
    if os.path.isdir(_p) and _p not in sys.path:
        sys.path.append(_p)

import ml_dtypes

B, S, D = 4, 4096, 1024
H, HD, O = 16, 64, 64
C = 128                  # chunk length
NCORE = 8
HPC = H // NCORE         # heads per core
NCHUNK = S // C

USE_BF16 = True

_CACHE = {}


def _build_program(nchunk=NCHUNK):
    import concourse.mybir as mybir
    from concourse import bacc
    from concourse.tile import TileContext

    fp32 = mybir.dt.float32
    cdt = mybir.dt.bfloat16 if USE_BF16 else fp32
    Alu = mybir.AluOpType
    Act = mybir.ActivationFunctionType

    nc = bacc.Bacc()
    # x interleaved per pair: [pr, 128, nchunk, 2, C] (chunk-major, batch
    # within chunk) so one matmul's moving operand covers both batches.
    xT_h = nc.declare_dram_parameter("xT", [2, 128, nchunk * 2 * C], cdt,
                                     isOutput=False)
    wq_h = nc.declare_dram_parameter("wq", [128, 128], cdt, isOutput=False)
    wk_h = nc.declare_dram_parameter("wk", [128, 128], cdt, isOutput=False)
    wv_h = nc.declare_dram_parameter("wv", [128, 128], cdt, isOutput=False)
    mask_h = nc.declare_dram_parameter("mask2", [128, 512], cdt, isOutput=False)
    ident_h = nc.declare_dram_parameter("ident", [128, 128], cdt, isOutput=False)
    ones_h = nc.declare_dram_parameter("ones", [1, 512], cdt, isOutput=False)
    zer_h = nc.declare_dram_parameter("zer", [1, 512], cdt, isOutput=False)
    # output grouped 2 chunks (4 bodies) per DMA so each partition ships
    # 2080 contiguous bytes (big packets; 520B/partition DMAs run ~15GB/s)
    out_h = nc.declare_dram_parameter("out", [nchunk // 2, 128, 4 * 260], cdt,
                                      isOutput=True)

    NXQ = 8       # x streamed in slices per pair for early start
    WORKBUFS = 5  # work pool depth (also gates the one-time vones memsets)

    with TileContext(nc) as tc:
        with (
            tc.tile_pool(name="consts", bufs=1) as consts,
            tc.tile_pool(name="work", bufs=WORKBUFS) as work,
            tc.tile_pool(name="st_sb", bufs=2) as st_sb,
            tc.tile_pool(name="osb", bufs=4) as osb,
            tc.tile_pool(name="pu", bufs=2, space="PSUM") as pu,
            tc.tile_pool(name="pa", bufs=1, space="PSUM") as pa,
            tc.tile_pool(name="pvk", bufs=1, space="PSUM") as pvk,
            tc.tile_pool(name="pnum", bufs=1, space="PSUM") as pnum,
            tc.tile_pool(name="pst", bufs=1, space="PSUM") as pst,
        ):
            # ---- constants + x into SBUF ----
            # Issue order matters: the first x slices and the weights gate
            # the first body, so they go before the other consts.
            wq = consts.tile([128, 128], cdt)
            wk = consts.tile([128, 128], cdt)
            wv = consts.tile([128, 128], cdt)
            mask2 = consts.tile([128, 512], cdt)
            ident = consts.tile([128, 128], cdt)
            ones = consts.tile([1, 512], cdt)
            zer = consts.tile([1, 512], cdt)
            xsb = [consts.tile([128, nchunk * 2 * C], cdt, name=f"xsb{pr}")
                   for pr in range(2)]
            qn = nchunk * 2 * C // NXQ
            nc.sync.dma_start(xsb[0][:, 0:qn], xT_h[0, :, 0:qn])
            nc.sync.dma_start(wv, wv_h[:, :])
            nc.sync.dma_start(wq, wq_h[:, :])
            nc.sync.dma_start(wk, wk_h[:, :])
            nc.sync.dma_start(xsb[1][:, 0:qn], xT_h[1, :, 0:qn])
            nc.sync.dma_start(mask2, mask_h[:, :])
            nc.sync.dma_start(ident, ident_h[:, :])
            nc.sync.dma_start(ones, ones_h[:, :])
            nc.sync.dma_start(zer, zer_h[:, :])
            for q in range(1, NXQ):
                for pr in range(2):
                    nc.sync.dma_start(xsb[pr][:, q * qn:(q + 1) * qn],
                                      xT_h[pr, :, q * qn:(q + 1) * qn])

            # persistent per-pair state PSUM: each [128, 260] with, per
            # j-block of 130 cols, head blocks [64h:64h+64, 65hp:65hp+65]
            # (hp = h ^ j).  One-time zero write sets data AND has_written
            # bits so later state matmuls accumulate; off-diag stays 0.
            st_ps = [
                pst.tile([128, 260], fp32, name="stA"),
                pst.tile([128, 260], fp32, name="stB"),
            ]
            for stp in st_ps:
                nc.tensor.matmul(stp, ones[:, 0:128], zer[:, 0:260],
                                 start=True, stop=False, skip_group_check=True)

            s01_prev = [None, None]

            for i in range(nchunk):
                csl = slice(i * 2 * C, (i + 1) * 2 * C)       # both batches
                bsl = [slice(i * 2 * C + j * C, i * 2 * C + (j + 1) * C)
                       for j in range(2)]                     # per batch

                # ---- v projections + V'aug for BOTH pairs (fused) ----
                # vk (1 bank): [v'(b0)|v'(b1)|v'(b2)|v'(b3)].  v needs only
                # x, so these dependency-free matmuls also fill PE bubbles
                # at chunk starts; the fused vaug copy halves its fixed cost.
                vk = pvk.tile([128, 512], fp32, name="vk")
                for vpr in range(2):
                    for j in range(2):
                        n = 2 * vpr + j
                        nc.tensor.matmul(vk[:, 128 * n:128 * (n + 1)],
                                         xsb[vpr][:, bsl[j]], wv,
                                         start=(n == 0), stop=(n == 3),
                                         skip_group_check=True)
                vaug = work.tile([128, 520], cdt, name="vaug")
                vsrc = vk.rearrange("p (g c) -> p g c", c=64)
                vdst = vaug.rearrange("p (g c) -> p g c", c=65)[:, :, 0:64]
                nc.scalar.copy(vdst, vsrc)
                if i < WORKBUFS:
                    vones = vaug.rearrange(
                        "p (g c) -> p g c", c=65)[:, :, 64:65]
                    nc.gpsimd.memset(vones, 1.0)

                for pr in range(2):
                    stp = st_ps[pr]
                    xp = xsb[pr]
                    V = 260 * pr     # this pair's vaug column offset

                    # ---------------- PE: projections ----------------
                    # u layout: [q(b0) | q(b1) | k(b0) | k(b1)] each [128,128]
                    u = pu.tile([128, 512], fp32, name="u")
                    nc.tensor.matmul(u[:, 0:256], wq, xp[:, csl],
                                     start=True, stop=False,
                                     skip_group_check=True)
                    nc.tensor.matmul(u[:, 256:512], wk, xp[:, csl],
                                     start=False, stop=True,
                                     skip_group_check=True)

                    # ------- phi = elu(u)+1 = min(max(u+1, 1), exp(u)) -------
                    # (identical: u<=0 -> exp(u); u>0 -> u+1 since exp>=u+1)
                    # max(u+1,1) depends only on u, so it runs concurrently
                    # with the scalar-engine exp instead of after it.
                    e2 = work.tile([128, 512], cdt, name="e2")
                    nc.scalar.activation(e2, u, Act.Exp)
                    w2 = work.tile([128, 512], cdt, name="w2")
                    nc.vector.tensor_scalar(w2, u, 1.0, 1.0, Alu.add, Alu.max)
                    phi2 = work.tile([128, 512], cdt, name="phi2")
                    nc.vector.tensor_tensor(phi2, w2, e2, Alu.min)

                    # ---------------- A^T = K Q^T per (b, h) ----------------
                    # One PSUM bank per head: all matmuls writing a given bank
                    # must read operands from the same base partition (HW).
                    # Bank A additionally hosts the k-transpose region (bf16
                    # via bitcast) — its writers also read base-0 operands,
                    # and their start=True bit-clears don't disturb ah data.
                    bankA = pa.tile([128, 512], fp32, name="bankA")
                    bankB = pa.tile([128, 512], fp32, name="bankB")
                    ah = [bankA[:, 0:256], bankB[:, 0:256]]
                    knp = bankA[:, 256:384].bitcast(cdt)   # [128, 256] bf16
                    for j in range(2):
                        qq = phi2[:, 128 * j:128 * (j + 1)]
                        kk = phi2[:, 256 + 128 * j:256 + 128 * (j + 1)]
                        for h in range(2):
                            es = slice(64 * h, 64 * (h + 1))
                            nc.tensor.matmul(
                                ah[h][:, 128 * j:128 * (j + 1)],
                                kk[es, :], qq[es, :],
                                start=(j == 0), stop=(j == 1),
                                skip_group_check=True)

                    # masked A -> SBUF; layout [b0h0 | b1h0 | b0h1 | b1h1]
                    am2 = work.tile([128, 512], cdt, name="am2")
                    nc.vector.tensor_tensor(am2[:, 0:256], ah[0],
                                            mask2[:, 0:256], Alu.mult)
                    nc.vector.tensor_tensor(am2[:, 256:512], ah[1],
                                            mask2[:, 256:512], Alu.mult)

                    # k transposed to [t, e] per j via PE transpose (skipped
                    # on the final chunk — nothing consumes the state after)
                    if i < nchunk - 1:
                        for j in range(2):
                            nc.tensor.transpose(
                                knp[:, 128 * j:128 * (j + 1)],
                                phi2[:, 256 + 128 * j:256 + 128 * (j + 1)],
                                ident)
                        knat2 = work.tile([128, 256], cdt, name="knat2")
                        nc.vector.tensor_copy(knat2, knp)

                    # ---------------- num = A_m^T Vaug + Q^T S ----------------
                    # For b1, head blocks are stored swapped (h1 first) so the
                    # h1 state block (output partition offset 64) lands at a
                    # column where its AP stays within one PSUM bank. Heads
                    # are summed at the end, so block identity is positional.
                    num = pnum.tile([128, 260], fp32, name="num")
                    first = True
                    for j in range(2):
                        for h in range(2):
                            hp = h ^ j  # head's positional slot
                            reg = slice(130 * j + 65 * hp,
                                        130 * j + 65 * (hp + 1))
                            va = vaug[:, V + 130 * j + 65 * h:
                                      V + 130 * j + 65 * (h + 1)]
                            nc.tensor.matmul(
                                num[:, reg],
                                am2[:, 256 * h + 128 * j:256 * h + 128 * (j + 1)],
                                va, start=first,
                                stop=(i == 0 and j == 1 and h == 1),
                                skip_group_check=True)
                            first = False
                        if i > 0:
                            # both heads at once: K=128 with block-diag state
                            sp = s01_prev[pr]
                            nc.tensor.matmul(
                                num[:, 130 * j:130 * (j + 1)],
                                phi2[:, 128 * j:128 * (j + 1)],
                                sp[:, 130 * j:130 * (j + 1)],
                                start=False, stop=(j == 1),
                                skip_group_check=True)

                    # ---------------- emit num/den to HBM (host divides) -----
                    # Emitted before the state update so the scalar queue
                    # frees the num bank early (the next body's AV matmuls
                    # wait on it); 4 bodies staged per osb tile, one fat DMA
                    # per 2 chunks.
                    if i % 2 == 0 and pr == 0:
                        numsb = osb.tile([128, 4 * 260], cdt, name="numsb")
                    off = ((i % 2) * 2 + pr) * 260
                    nc.scalar.copy(numsb[:, off:off + 260], num)
                    if i % 2 == 1 and pr == 1:
                        nc.sync.dma_start(out_h[i // 2], numsb)

                    # ---------------- state update (diag blocks only) --------
                    # Per-head matmuls with base-0 operands; h1 writes at
                    # output partition offset 64. Off-diag blocks stay zero.
                    # Skipped on the final chunk (state is never read again).
                    if i < nchunk - 1:
                        for j in range(2):
                            for h in range(2):
                                hp = h ^ j
                                nc.tensor.matmul(
                                    stp[64 * h:64 * (h + 1),
                                        130 * j + 65 * hp:
                                        130 * j + 65 * (hp + 1)],
                                    knat2[:, 128 * j + 64 * h:
                                          128 * j + 64 * (h + 1)],
                                    vaug[:, V + 130 * j + 65 * h:
                                         V + 130 * j + 65 * (h + 1)],
                                    start=False, stop=False,
                                    skip_group_check=True)

                        s01 = st_sb.tile([128, 260], cdt, name="s01")
                        nc.scalar.copy(s01, stp)
                        s01_prev[pr] = s01

    nc.finalize()
    return nc


def _host_prep(x, Wq, Wk, Wv, Wp):
    """Shard inputs per core; returns in_maps list."""
    x = np.asarray(x, dtype=np.float32)
    Wq = np.asarray(Wq, dtype=np.float32)
    Wk = np.asarray(Wk, dtype=np.float32)
    Wv = np.asarray(Wv, dtype=np.float32)
    Wp = np.asarray(Wp, dtype=np.float32)
    ndt = ml_dtypes.bfloat16 if USE_BF16 else np.float32

    mask = np.triu(np.ones((C, C), np.float32))
    mask2 = np.tile(mask, (1, 4)).astype(ndt)          # [128, 512]
    ident = np.eye(128, dtype=np.float32).astype(ndt)
    ones = np.ones((1, 512), np.float32).astype(ndt)

    in_maps = []
    for c in range(NCORE):
        h0 = HPC * c
        xs = x[:, :, 64 * h0:64 * (h0 + HPC)]          # [B, S, 128]
        # [pr, 128, nchunk, 2, C]: chunk-major with the pair's two batches
        # interleaved per chunk.
        xT = xs.transpose(0, 2, 1).reshape(2, 2, 128, NCHUNK, C)
        xT = np.ascontiguousarray(xT.transpose(0, 2, 3, 1, 4)).reshape(
            2, 128, NCHUNK * 2 * C).astype(ndt)
        wq_bd = np.zeros((128, 128), np.float32)
        wk_bd = np.zeros((128, 128), np.float32)
        wv_bd = np.zeros((128, 128), np.float32)
        for j in range(HPC):
            h = h0 + j
            sl = slice(64 * j, 64 * (j + 1))
            wq_bd[sl, sl] = Wq[h]
            wk_bd[sl, sl] = Wk[h]
            wv_bd[sl, sl] = Wv[h] @ Wp[64 * h:64 * (h + 1), :]
        in_maps.append({
            "xT": xT,
            "wq": wq_bd.astype(ndt),
            "wk": wk_bd.astype(ndt),
            "wv": wv_bd.astype(ndt),
            "mask2": mask2,
            "ident": ident,
            "ones": ones,
            "zer": np.zeros((1, 512), np.float32).astype(ndt),
        })
    return in_maps


def get_program():
    if "nc" not in _CACHE:
        _CACHE["nc"] = _build_program()
    return _CACHE["nc"]


def run_spmd(in_maps, **kwargs):
    from concourse.bass_utils import run_bass_kernel_spmd
    nc = get_program()
    return run_bass_kernel_spmd(nc, in_maps, list(range(NCORE)), **kwargs)


def kernel(x, Wq, Wk, Wv, Wp):
    in_maps = _host_prep(x, Wq, Wk, Wv, Wp)
    res = run_spmd(in_maps)
    out = np.zeros((B, S, O), np.float32)
    for c in range(NCORE):
        # raw [ngrp, t, il, pr, j, slot, col]: chunk i = 4*g + il; per
        # j-block two head slots of [64 num | 1 den]; head order within a
        # block is irrelevant because the heads are summed.
        raw = np.asarray(res.results[c]["out"], dtype=np.float32)
        raw = raw.reshape(NCHUNK // 2, 128, 2, 2, 2, 2, 65)
        y = (raw[..., 0:64] / raw[..., 64:65]).sum(axis=5)
        # y: [g, t, il, pr, j, 64] -> b = 2*pr + j, token 128*(2g+il) + t
        out += y.transpose(3, 4, 0, 2, 1, 5).reshape(B, S, O)
    return out
